# revision 1
# baseline (speedup 1.0000x reference)
"""Trainium2 Bass kernel for ModelToVolumeAligner (gaussian-splat volume + correlation loss).

Strategy:
  - Host: compute 3x3 rotation from quat (O(1)), assign atoms to y-bins
    (Gaussian support is ~±8 voxels since var<=2), shard atoms bin-balanced
    across the 8 cores, and build per-rank (atom,gaussian) layout arrays.
  - Device (per core): rotate positions (PE), build per-rank 1D gaussian
    factors W(z),U(y-window),V(x) via ACT Square/Exp, form Khatri-Rao
    products U⊗V (DVE), accumulate vol[z, y-window, x] slabs via PE matmuls
    over rank tiles, scatter slabs into a local volume (margin-padded in y).
  - ReduceScatter the partial volumes (bf16) across cores; each core reduces
    its z-shard against its voxel-grid shard -> 3 scalars (ssq, dot, gssq).
  - Host: combine 8x3 scalars -> 1 - dot/sqrt(ssq*gssq).
    (The v/v.sum() normalization in the reference cancels in the correlation.)
"""

import math
import numpy as np

import concourse.bass as bass
import concourse.mybir as mybir
import concourse.tile as tile
from concourse import bacc
from concourse.bass import ts
from concourse.bass_utils import run_bass_kernel_spmd

F32 = mybir.dt.float32
BF16 = mybir.dt.bfloat16
AF = mybir.ActivationFunctionType
OP = mybir.AluOpType

N_CORES = 8
P = 128
G = 5
NZ = NY = NX = 128
NBINS = 16
BINW = 8          # y-bin width in voxels
MARGIN = 4        # halo on each side of a bin (gaussian support)
WIN = BINW + 2 * MARGIN          # y window per bin (20)
OVL = (WIN - BINW) * NX          # slab cols overlapping previously-written vol
KR_GP = 0                        # KR columns computed on GPSIMD (rest on DVE)
D2_GP = False                    # compute d2z/d2x on GPSIMD instead of ACT
D2_DVE = False                   # compute d2z/d2x on DVE instead of ACT
KR_TT = 1                        # 0: per-column tensor_scalar; 1: one broadcast
                                 # tensor_tensor per tile; 2: broadcast TT
                                 # alternating DVE/GPSIMD by tile
KR_SPLIT = 0                     # >0: columns of the KR computed on GPSIMD
                                 # (rest on DVE), as two broadcast TTs
ISO_KR = None                    # isolation: override number of KR columns
ISO_P1 = False                   # isolation: skip rotation phase (memset biases)
ISO_HOIST = 0                    # 1: hoisted static tables (KR+MM only);
                                 # 2: also skip KR (MM only)
ACT5 = True                      # merged 5-op ACT path (exp over [d2z|d2x])
U_BF = True                      # u_t in bf16 (uniform dtypes in the KR TT)
KR_PARTS = 1                     # split the KR broadcast into N sub-ops for
                                 # finer-grained PE overlap
ISO_DRAIN = False                # isolation: shrink bin drains to 64 cols
ISO_MM = None                    # isolation: override number of matmuls/tile
ISO_ACT = True                   # isolation: include the big [128,128] ACT ops
WK_BUFS = 8                      # work pool depth
YPAD = NY + 2 * MARGIN           # 144 padded y size
SLAB = WIN * NX                  # 3072 free-dim cols of a bin slab
VOL_COLS = YPAD * NX             # 18432
AMP_PAD = 1e-30
ZSH = NZ // N_CORES              # 16 z-slices per core after reduce-scatter

_cache = {}


def _rotmat(quat):
    q = quat.astype(np.float32)
    q = q / np.sqrt((q * q).sum(dtype=np.float32))
    w, x, y, z = [np.float32(v) for v in q]
    return np.array(
        [
            [1 - 2 * (y * y + z * z), 2 * (x * y - w * z), 2 * (x * z + w * y)],
            [2 * (x * y + w * z), 1 - 2 * (x * x + z * z), 2 * (y * z - w * x)],
            [2 * (x * z - w * y), 2 * (y * z + w * x), 1 - 2 * (x * x + y * y)],
        ],
        dtype=np.float32,
    )


def _build_program(ktiles, tile_bin, n_cores=N_CORES, loop_reps=0,
                   with_collective=True):
    """Build the Bass/Tile program for a given per-bin tile structure.

    loop_reps>0 wraps the compute phases in a device-side loop (timing
    variant); with_collective=False replaces the ReduceScatter with a local
    copy (for single-core cost-model simulation).
    """
    K_TOT = sum(ktiles)
    R_PAD = K_TOT * P

    nc = bacc.Bacc("TRN2", target_bir_lowering=False, debug=False,
                   num_devices=n_cores)

    # ---- I/O ----
    posT_d = nc.dram_tensor("posT", [4, R_PAD], F32, kind="ExternalInput")
    var_d = nc.dram_tensor("var_t", [P, K_TOT], F32, kind="ExternalInput")
    amp_d = nc.dram_tensor("amp_t", [P, K_TOT], F32, kind="ExternalInput")
    rot_d = nc.dram_tensor("rot", [4, 3], F32, kind="ExternalInput")
    grid_d = nc.dram_tensor("gshard", [P, ZSH * NY * NX // P], F32,
                            kind="ExternalInput")
    out_d = nc.dram_tensor("res3", [3, 1], F32, kind="ExternalOutput")

    # ---- compile-time constants (embedded in NEFF) ----
    zc_np = np.broadcast_to((np.arange(NZ, dtype=np.float32) - NZ // 2),
                            (P, NZ)).copy()
    ar24_np = np.broadcast_to(np.arange(WIN, dtype=np.float32), (P, WIN)).copy()
    shift_np = np.broadcast_to(
        (BINW * np.asarray(tile_bin, np.float32) - (MARGIN + NY // 2)),
        (P, K_TOT)).copy()
    ones_np = np.ones((P, 1), np.float32)
    zc_c = nc.inline_tensor(zc_np, name="zc_c")
    ar24_c = nc.inline_tensor(ar24_np, name="ar24_c")
    shift_c = nc.inline_tensor(shift_np, name="shift_c")
    ones_c = nc.inline_tensor(ones_np, name="ones_c")

    with tile.TileContext(nc) as tc:
        with tc.tile_pool(name="keep", bufs=1) as keep:
            # persistent small tiles
            zc_sb = keep.tile([P, NZ], F32)
            nc.sync.dma_start(zc_sb[:], zc_c[:, :])
            ar24_sb = keep.tile([P, WIN], F32)
            nc.sync.dma_start(ar24_sb[:], ar24_c[:, :])
            ones_sb = keep.tile([P, 1], F32)
            nc.sync.dma_start(ones_sb[:], ones_c[:, :])
            var_sb = keep.tile([P, K_TOT], F32)
            nc.sync.dma_start(var_sb[:], var_d[:, :])
            amp_sb = keep.tile([P, K_TOT], F32)
            nc.sync.dma_start(amp_sb[:], amp_d[:, :])

            bias_z = keep.tile([P, K_TOT], F32)
            bias_y = keep.tile([P, K_TOT], F32)
            bias_x = keep.tile([P, K_TOT], F32)
            sc = keep.tile([P, K_TOT], F32)
            lnpref = keep.tile([P, K_TOT], F32)

            # ---- phase 1: rotate positions, derive per-rank scalars ----
            def phase12(vol):
                _phase12(nc, tc, K_TOT, ktiles, zc_sb, ar24_sb, var_sb,
                         amp_sb, bias_z, bias_y, bias_x, sc, lnpref,
                         posT_d, rot_d, shift_c, vol)

            with tc.tile_pool(name="volp", bufs=1) as volp:
                vol = volp.tile([P, VOL_COLS], F32)
                if loop_reps:
                    with tc.For_i(0, loop_reps, 1):
                        phase12(vol)
                else:
                    phase12(vol)

                # ---- phase 3: reduce-scatter + local reduction ----
                with tc.tile_pool(name="fin", bufs=1) as fin, \
                     tc.tile_pool(name="find", bufs=1, space="DRAM") as find, \
                     tc.tile_pool(name="finps", bufs=1, space="PSUM") as finps:
                    vol_bf = fin.tile([P, NZ * NY * NX // P], BF16)
                    nc.vector.tensor_copy(
                        vol_bf[:], vol[:, MARGIN * NX:MARGIN * NX + NY * NX])
                    cc_in = find.tile([P, NY * NX], BF16)
                    nc.sync.dma_start(cc_in[:], vol_bf[:])
                    cc_out = find.tile([ZSH, NY * NX], BF16)
                    if with_collective:
                        nc.gpsimd.collective_compute(
                            "ReduceScatter", OP.add,
                            replica_groups=[list(range(n_cores))],
                            ins=[cc_in[:, :].opt()], outs=[cc_out[:, :].opt()])
                    else:
                        nc.sync.dma_start(cc_out[:, :], cc_in[:ZSH, :])
                    vsh = fin.tile([P, ZSH * NY * NX // P], BF16)
                    nc.sync.dma_start(
                        vsh[:],
                        cc_out[:, :].rearrange("a (c f) -> (a c) f", c=P // ZSH))
                    vshf = fin.tile([P, ZSH * NY * NX // P], F32)
                    nc.vector.tensor_copy(vshf[:], vsh[:])
                    gsh = fin.tile([P, ZSH * NY * NX // P], F32)
                    nc.sync.dma_start(gsh[:], grid_d[:, :])

                    scratch = fin.tile([P, ZSH * NY * NX // P], F32)
                    parts = fin.tile([P, 3], F32)
                    for j, (a, b2) in enumerate(
                            [(vshf, vshf), (vshf, gsh), (gsh, gsh)]):
                        nc.vector.tensor_tensor(out=scratch[:], in0=a[:],
                                                in1=b2[:], op=OP.mult)
                        nc.vector.tensor_reduce(
                            out=parts[:, j:j + 1], in_=scratch[:],
                            axis=mybir.AxisListType.X, op=OP.add)
                    red = finps.tile([3, 1], F32)
                    nc.tensor.matmul(out=red[:], lhsT=parts[:, 0:3],
                                     rhs=ones_sb[:, :], start=True, stop=True)
                    red_sb = fin.tile([3, 1], F32)
                    nc.scalar.copy(red_sb[:], red[:])
                    nc.sync.dma_start(out_d[:, :], red_sb[:])

    nc.compile()
    return nc


def _phase12(nc, tc, K_TOT, ktiles, zc_sb, ar24_sb, var_sb, amp_sb,
             bias_z, bias_y, bias_x, sc, lnpref,
             posT_d, rot_d, shift_c, vol):
    R_PAD = K_TOT * P
    if ISO_P1:
        nc.vector.memset(bias_z[:], 0.0)
        nc.vector.memset(bias_x[:], 0.0)
        nc.vector.memset(bias_y[:], -10.0)
        nc.vector.memset(sc[:], -0.5)
        nc.vector.memset(lnpref[:], -2.0)
    if not ISO_P1:
      with tc.tile_pool(name="rotp", bufs=1) as rotp, \
         tc.tile_pool(name="rotps", bufs=2, space="PSUM") as rotps:
        posT_sb = rotp.tile([4, R_PAD], F32)
        nc.sync.dma_start(posT_sb[:], posT_d[:, :])
        rot_sb = rotp.tile([4, 3], F32)
        nc.sync.dma_start(rot_sb[:], rot_d[:, :])
        shift_sb = rotp.tile([P, K_TOT], F32)
        nc.sync.dma_start(shift_sb[:], shift_c[:, :])

        posr = rotp.tile([P, K_TOT, 3], F32)
        GRP = 16
        for t0g in range(0, K_TOT, GRP):
            gn = min(GRP, K_TOT - t0g)
            ps = rotps.tile([P, 3 * GRP], F32, tag="rps", bufs=2)
            for j in range(gn):
                t = t0g + j
                nc.tensor.matmul(out=ps[:, 3 * j:3 * j + 3],
                                 lhsT=posT_sb[:, ts(t, P)],
                                 rhs=rot_sb[:, :], start=True, stop=True)
            nc.scalar.copy(posr[:, t0g:t0g + gn, :], ps[:, :3 * gn])

        nc.vector.tensor_scalar_mul(bias_z[:], posr[:, :, 2], -1.0)
        nc.vector.tensor_scalar_mul(bias_x[:], posr[:, :, 0], -1.0)
        nc.vector.tensor_tensor(out=bias_y[:], in0=shift_sb[:],
                                in1=posr[:, :, 1], op=OP.subtract)
        inv = rotp.tile([P, K_TOT], F32)
        nc.vector.reciprocal(inv[:], var_sb[:])
        nc.vector.tensor_scalar_mul(sc[:], inv[:], -0.5)
        lnv = rotp.tile([P, K_TOT], F32)
        nc.scalar.activation(lnv[:], var_sb[:], AF.Ln,
                             scale=float(2 * math.pi))
        lnam = rotp.tile([P, K_TOT], F32)
        nc.scalar.activation(lnam[:], amp_sb[:], AF.Ln)
        nc.vector.tensor_scalar_mul(lnv[:], lnv[:], -1.5)
        nc.vector.tensor_tensor(out=lnpref[:], in0=lnam[:],
                                in1=lnv[:], op=OP.add)

    # ---- phase 2: main splat loop ----
    nc.gpsimd.memset(vol[:], 0.0)
    with tc.tile_pool(name="work", bufs=3) as wk, \
         tc.tile_pool(name="slabp", bufs=1, space="PSUM") as slabp:
        if ISO_HOIST:
            hoist_w = wk.tile([P, NZ], BF16, name="hoist_w")
            nc.gpsimd.memset(hoist_w[:], 0.25)
            hoist_v = wk.tile([P, NX], BF16, name="hoist_v")
            nc.gpsimd.memset(hoist_v[:], 0.25)
            hoist_u = wk.tile([P, WIN], F32, name="hoist_u")
            nc.gpsimd.memset(hoist_u[:], 0.25)
            hoist_kr = wk.tile([P, SLAB], BF16, name="hoist_kr")
            nc.gpsimd.memset(hoist_kr[:], 0.0625)
        t0 = 0
        for b in range(NBINS):
            nt = ktiles[b]
            if nt == 0:
                continue
            slab = slabp.tile([P, SLAB], F32, tag="slab", bufs=2)
            for i in range(nt):
                t = t0 + i
                if ISO_HOIST:
                    w_t, v_t, u_t = hoist_w, hoist_v, hoist_u
                elif ACT5:
                    d2zx = wk.tile([P, 2 * NZ], F32, tag="d2zx", bufs=WK_BUFS)
                    nc.scalar.activation(d2zx[:, :NZ], zc_sb[:], AF.Square,
                                         bias=bias_z[:, t:t + 1])
                    nc.scalar.activation(d2zx[:, NZ:], zc_sb[:], AF.Square,
                                         bias=bias_x[:, t:t + 1])
                    wv = wk.tile([P, 2 * NZ], BF16, tag="wv", bufs=WK_BUFS)
                    nc.scalar.activation(wv[:], d2zx[:], AF.Exp,
                                         scale=sc[:, t:t + 1])
                    w_t = wv[:, :NZ]
                    v_t = wv[:, NZ:]
                    d2y = wk.tile([P, WIN], F32, tag="d2y", bufs=WK_BUFS)
                    nc.scalar.activation(d2y[:], ar24_sb[:], AF.Square,
                                         bias=bias_y[:, t:t + 1])
                    u_t = wk.tile([P, WIN], BF16 if U_BF else F32,
                                  tag="u_t", bufs=WK_BUFS)
                    nc.scalar.activation(u_t[:], d2y[:], AF.Exp,
                                         scale=sc[:, t:t + 1],
                                         bias=lnpref[:, t:t + 1])
                else:
                    d2z = (wk.tile([P, NZ], F32, tag="d2z", bufs=WK_BUFS,
                                   name="d2z") if ISO_ACT else None)
                    if not ISO_ACT:
                        pass
                    elif D2_GP:
                        nc.gpsimd.tensor_scalar_add(d2z[:], zc_sb[:],
                                                    bias_z[:, t:t + 1])
                        nc.gpsimd.tensor_mul(d2z[:], d2z[:], d2z[:])
                    elif D2_DVE:
                        nc.vector.tensor_scalar_add(d2z[:], zc_sb[:],
                                                    bias_z[:, t:t + 1])
                        nc.vector.tensor_tensor(out=d2z[:], in0=d2z[:],
                                                in1=d2z[:], op=OP.mult)
                    else:
                        nc.scalar.activation(d2z[:], zc_sb[:], AF.Square,
                                             bias=bias_z[:, t:t + 1])
                    w_t = wk.tile([P, NZ], BF16, tag="w_t", bufs=WK_BUFS)
                    if ISO_ACT:
                        nc.scalar.activation(w_t[:], d2z[:], AF.Exp,
                                             bias=lnpref[:, t:t + 1],
                                             scale=sc[:, t:t + 1])
                    else:
                        nc.gpsimd.memset(w_t[:], 0.25)
                    d2x = (wk.tile([P, NX], F32, tag="d2x", bufs=WK_BUFS,
                                   name="d2x") if ISO_ACT else None)
                    if not ISO_ACT:
                        pass
                    elif D2_GP:
                        nc.gpsimd.tensor_scalar_add(d2x[:], zc_sb[:],
                                                    bias_x[:, t:t + 1])
                        nc.gpsimd.tensor_mul(d2x[:], d2x[:], d2x[:])
                    elif D2_DVE:
                        nc.vector.tensor_scalar_add(d2x[:], zc_sb[:],
                                                    bias_x[:, t:t + 1])
                        nc.vector.tensor_tensor(out=d2x[:], in0=d2x[:],
                                                in1=d2x[:], op=OP.mult)
                    else:
                        nc.scalar.activation(d2x[:], zc_sb[:], AF.Square,
                                             bias=bias_x[:, t:t + 1])
                    v_t = wk.tile([P, NX], BF16, tag="v_t", bufs=WK_BUFS)
                    if ISO_ACT:
                        nc.scalar.activation(v_t[:], d2x[:], AF.Exp,
                                             scale=sc[:, t:t + 1])
                    else:
                        nc.gpsimd.memset(v_t[:], 0.25)
                    d2y = wk.tile([P, WIN], F32, tag="d2y", bufs=WK_BUFS)
                    nc.scalar.activation(d2y[:], ar24_sb[:], AF.Square,
                                         bias=bias_y[:, t:t + 1])
                    u_t = wk.tile([P, WIN], F32, tag="u_t", bufs=WK_BUFS)
                    nc.scalar.activation(u_t[:], d2y[:], AF.Exp,
                                         scale=sc[:, t:t + 1])
                if ISO_HOIST == 2:
                    kr = hoist_kr
                elif True:
                    kr = wk.tile([P, SLAB], BF16, tag="kr", bufs=WK_BUFS)
                if ISO_HOIST == 2:
                    pass
                elif KR_SPLIT:
                    nd = WIN - KR_SPLIT
                    kr3 = kr[:].rearrange("p (w x) -> p w x", x=NX)
                    nc.vector.tensor_tensor(
                        out=kr3[:, :nd, :],
                        in0=u_t[:, :nd].unsqueeze(2).to_broadcast([P, nd, NX]),
                        in1=v_t[:].unsqueeze(1).to_broadcast([P, nd, NX]),
                        op=OP.mult)
                    nc.gpsimd.tensor_tensor(
                        out=kr3[:, nd:, :],
                        in0=u_t[:, nd:].unsqueeze(2).to_broadcast(
                            [P, KR_SPLIT, NX]),
                        in1=v_t[:].unsqueeze(1).to_broadcast(
                            [P, KR_SPLIT, NX]),
                        op=OP.mult)
                elif KR_TT:
                    eng = nc.vector
                    if KR_TT == 2 and (t % 3 == 2):
                        eng = nc.gpsimd
                    kr3 = kr[:].rearrange("p (w x) -> p w x", x=NX)
                    wp = WIN // KR_PARTS
                    for q in range(KR_PARTS):
                        eng.tensor_tensor(
                            out=kr3[:, q * wp:(q + 1) * wp, :],
                            in0=u_t[:, q * wp:(q + 1) * wp].unsqueeze(2)
                                .to_broadcast([P, wp, NX]),
                            in1=v_t[:].unsqueeze(1).to_broadcast([P, wp, NX]),
                            op=OP.mult)
                else:
                    ncols = WIN if ISO_KR is None else ISO_KR
                    for w in range(ncols):
                        eng = nc.gpsimd if w < KR_GP else nc.vector
                        eng.tensor_scalar_mul(
                            kr[:, ts(w, NX)], v_t[:], u_t[:, w:w + 1])
                nmm = (SLAB // 512) if ISO_MM is None else ISO_MM
                for n in range(nmm):
                    nc.tensor.matmul(
                        out=slab[:, ts(n, 512)], lhsT=w_t[:],
                        rhs=kr[:, ts(n, 512)],
                        start=(i == 0), stop=(i == nt - 1))
            # scatter slab into vol: window cols [8b, 8b+WIN) of ypad
            base = BINW * b * NX
            if ISO_DRAIN:
                nc.vector.tensor_tensor(
                    out=vol[:, base:base + 64],
                    in0=vol[:, base:base + 64],
                    in1=slab[:, :64], op=OP.add)
            else:
                nc.vector.tensor_tensor(
                    out=vol[:, base:base + OVL],
                    in0=vol[:, base:base + OVL],
                    in1=slab[:, :OVL], op=OP.add)
                nc.scalar.copy(vol[:, base + OVL:base + SLAB],
                               slab[:, OVL:])
            t0 += nt


def _prepare(quat, offset, positions, amplitudes, variances, voxel_grid):
    quat = np.asarray(quat, np.float32)
    offset = np.asarray(offset, np.float32)
    positions = np.asarray(positions, np.float32)
    amplitudes = np.asarray(amplitudes, np.float32)
    variances = np.asarray(variances, np.float32)
    voxel_grid = np.asarray(voxel_grid, np.float32)

    rot = _rotmat(quat)
    # y coordinate (in voxel-index space) for bin assignment only
    py = positions @ rot[:, 1] + offset[1] + np.float32(NY // 2)
    bins = np.clip(np.floor(py / BINW).astype(np.int64), 0, NBINS - 1)

    # global per-bin lists, bin-balanced core assignment
    bin_idxs = [np.nonzero(bins == b)[0] for b in range(NBINS)]
    caps = [int(math.ceil(len(ix) / N_CORES)) if len(ix) else 0
            for ix in bin_idxs]
    ktiles = [int(math.ceil(c * G / P)) if c else 0 for c in caps]
    K_TOT = sum(ktiles)
    R_PAD = K_TOT * P
    tile_bin = []
    for b in range(NBINS):
        tile_bin += [b] * ktiles[b]

    # per-core input construction
    in_maps = []
    for c in range(N_CORES):
        atom_of_rank = np.full(R_PAD, -1, np.int64)
        g_of_rank = np.zeros(R_PAD, np.int64)
        r0 = 0
        for b in range(NBINS):
            if ktiles[b] == 0:
                continue
            mine = bin_idxs[b][c::N_CORES]
            n = len(mine)
            ranks = np.arange(n * G)
            atom_of_rank[r0 + ranks] = np.repeat(mine, G)
            g_of_rank[r0 + ranks] = np.tile(np.arange(G), n)
            r0 += ktiles[b] * P
        valid = atom_of_rank >= 0
        av = np.where(valid, atom_of_rank, 0)

        posT = np.where(valid[None, :], positions[av].T, np.float32(0.0))
        posT = np.ascontiguousarray(
            np.concatenate([posT, np.ones((1, R_PAD), np.float32)], axis=0))
        var_r = np.where(valid, variances[av, g_of_rank], np.float32(1.0))
        amp_r = np.where(valid, amplitudes[av, g_of_rank], np.float32(AMP_PAD))
        var_t = np.ascontiguousarray(var_r.reshape(K_TOT, P).T, np.float32)
        amp_t = np.ascontiguousarray(amp_r.reshape(K_TOT, P).T, np.float32)
        gshard = np.ascontiguousarray(
            voxel_grid[c * ZSH:(c + 1) * ZSH].reshape(P, ZSH * NY * NX // P))
        in_maps.append({
            "posT": posT,
            "var_t": var_t,
            "amp_t": amp_t,
            "rot": np.concatenate([rot, offset[None, :]], axis=0),
            "gshard": gshard,
        })
    return ktiles, tile_bin, in_maps


def _combine(results):
    ssq = dot = gssq = 0.0
    for c in range(N_CORES):
        r = results[c]["res3"]
        ssq += float(r[0, 0])
        dot += float(r[1, 0])
        gssq += float(r[2, 0])
    corr = dot / math.sqrt(ssq * gssq)
    return np.float32(1.0 - corr)


def kernel(quat, offset, positions, amplitudes, variances, voxel_grid):
    ktiles, tile_bin, in_maps = _prepare(
        quat, offset, positions, amplitudes, variances, voxel_grid)
    key = (tuple(ktiles),)
    if key not in _cache:
        _cache[key] = _build_program(ktiles, tile_bin)
    nc = _cache[key]
    res = run_bass_kernel_spmd(nc, in_maps, core_ids=list(range(N_CORES)))
    return _combine(res.results)

